# revision 23
# baseline (speedup 1.0000x reference)
"""Trainium2 Bass kernel for nn_DependencyEncoder (stack TreeLSTM).

Self-contained: takes FULL inputs as in reference.setup_inputs(), shards the
batch across 8 NeuronCores (pure data parallelism), runs a fully static
Bass/Tile program specialized on the (batch-uniform) transition schedule,
and gathers the full [B, H] output.

Device program layout (per core, b = B/8 examples):
- Everything feature-on-partition, batch on the free dim.
- tokens_h^T / tokens_c^T as [128, 2*L*b], free = h1*(L*b) + t*b + e.
- Track gates row-permuted to (i, f, o, 2g); PSUM [128, 2b]: chunk0=[i;f],
  chunk1=[o;2g].  tanh(g) computed as 2*sigmoid(2g)-1 (one sigmoid op over
  all four gates; the 2x is pre-folded into the weights).
- Tree gates (i, o, f_l, f_r, 2u) in PSUM [128, 10b], one sigmoid op.
- Biases ride augmented matmuls: th state tile is [TD+1, b] with last row 1,
  multiplied by [W_hh^T; b_ih+b_hh] and [W_x^T; b_l].
"""

import os
import sys

os.environ.setdefault("JAX_PLATFORMS", "")
if "/opt/trn_rl_repo" not in sys.path:
    sys.path.insert(0, "/opt/trn_rl_repo")

import numpy as np
import ml_dtypes

BF16 = ml_dtypes.bfloat16
N_CORES = 8
H = 256
TD = 64

# ---------------------------------------------------------------- schedule --

# Track gate rows: original (i, f, g, o); device order (i, 2g, f, o) so the
# sigmoid can split [i,2g | f,o] and the tanh-path DVE work starts after the
# first half.
_TRACK_PERM = np.concatenate([np.arange(0, 64), np.arange(128, 192),
                              np.arange(64, 128), np.arange(192, 256)])
_TRACK_SCALE = np.concatenate([np.ones(64), np.full(64, 2.0),
                               np.ones(128)]).astype(np.float32)
# Tree gate rows reordered to (u, i, o, f_l, f_r); u rows first, scaled x2.
_TREE_PERM = np.concatenate([np.arange(4 * H, 5 * H), np.arange(0, 4 * H)])
_TREE_SCALE = np.concatenate([np.full(H, 2.0), np.ones(4 * H)]).astype(np.float32)


def derive_schedule(transitions: np.ndarray, L: int):
    """Symbolic stack simulation over the batch-uniform transition codes."""
    tr = np.asarray(transitions)
    if not (tr == tr[0:1]).all():
        raise NotImplementedError("non-batch-uniform transitions unsupported")
    codes = [int(c) for c in tr[0]]
    MAX_STACK = L + 2
    stack = [("tok", 0), ("tok", 0)] + [None] * (MAX_STACK - 2)
    p, bp, nred = 2, 0, 0
    steps = []
    for c in codes:
        assert 2 <= p <= MAX_STACK, f"invalid stack pointer {p}"
        top = stack[p - 1]
        sec = stack[p - 2]
        buf = ("tok", min(bp, L - 1))
        is_shift = c == 1
        is_red = c in (2, 3)
        step = dict(code=c, buf=buf, top=top, sec=sec, is_red=is_red,
                    head=None, chil=None, red_idx=None)
        if is_red:
            head, chil = (top, sec) if c == 2 else (sec, top)
            val = ("red", nred)
            step.update(head=head, chil=chil, red_idx=nred)
            nred += 1
        elif is_shift:
            val = buf
        else:
            val = top
        pos = p if is_shift else (p - 2 if is_red else p - 1)
        assert 0 <= pos < MAX_STACK
        stack[pos] = val
        p = p + int(is_shift) - int(is_red)
        bp = bp + int(is_shift)
        steps.append(step)
    return steps, stack[p - 1]


# ------------------------------------------------------------ host packing --

def _chunk_k(wt: np.ndarray) -> np.ndarray:
    """[K, M] -> [128, (K//128)*M], K-chunks stacked along the free dim."""
    K = wt.shape[0]
    assert K % 128 == 0
    return np.hstack([wt[k * 128:(k + 1) * 128] for k in range(K // 128)])


def prep_weights(W_x, U_r, U_l, b_l, W_ih, W_hh, b_ih, b_hh):
    W_ih = np.asarray(W_ih, np.float32)
    W_hh = np.asarray(W_hh, np.float32)
    sc = _TRACK_SCALE[:, None]
    W_A = W_ih[:, 0:H][_TRACK_PERM] * sc
    W_B = W_ih[:, H:2 * H][_TRACK_PERM] * sc
    W_C = W_ih[:, 2 * H:3 * H][_TRACK_PERM] * sc
    W_hh_p = W_hh[_TRACK_PERM] * sc
    btot = ((np.asarray(b_ih) + np.asarray(b_hh))[_TRACK_PERM] * _TRACK_SCALE)

    tsc = _TREE_SCALE[:, None]
    U_l_p = np.asarray(U_l, np.float32)[_TREE_PERM] * tsc
    U_r_p = np.asarray(U_r, np.float32)[_TREE_PERM] * tsc
    W_x_p = np.asarray(W_x, np.float32)[_TREE_PERM] * tsc
    b_l_p = np.asarray(b_l, np.float32)[_TREE_PERM] * _TREE_SCALE
    out = dict(
        wa=_chunk_k(np.ascontiguousarray(W_A.T)),                    # [128, 512]
        wb=_chunk_k(np.ascontiguousarray(W_B.T)),
        wc=_chunk_k(np.ascontiguousarray(W_C.T)),
        whh=np.vstack([W_hh_p.T, btot[None, :]]),                    # [65, 256]
        ul=_chunk_k(np.ascontiguousarray(U_l_p.T)),
        ur=_chunk_k(np.ascontiguousarray(U_r_p.T)),
        wx=np.vstack([W_x_p.T, b_l_p[None, :]]),                     # [65, 1280]
        ident=np.eye(128, dtype=np.float32),
    )
    return {k: np.ascontiguousarray(v, dtype=BF16) for k, v in out.items()}


def prep_tokens(tokens: np.ndarray, dtype=np.float32) -> np.ndarray:
    """[b, L, H] -> [128, 2*L*b], free = h1*(L*b) + t*b + e."""
    b, L, Hn = tokens.shape
    assert Hn == H
    arr = np.asarray(tokens, np.float32).transpose(2, 1, 0).reshape(H, L * b)
    return np.ascontiguousarray(np.hstack([arr[:128], arr[128:]]).astype(dtype))


# ---------------------------------------------------------- device program --

def _build_program(steps, out_sym, b, L, debug_taps=False, n_dummy=0):
    import concourse.bacc as bacc
    import concourse.mybir as mybir
    import concourse.tile as tile

    f32 = mybir.dt.float32
    bf16 = mybir.dt.bfloat16
    AF = mybir.ActivationFunctionType
    nc = bacc.Bacc("TRN2", target_bir_lowering=False, debug=False)
    Lb = L * b

    d = {}
    DTYPES = {"tokc": f32, "tc0": f32}
    for name, shape in [
        ("tokh", [128, 2 * Lb]), ("tokc", [128, 2 * Lb]),
        ("wa", [128, 512]), ("wb", [128, 512]), ("wc", [128, 512]),
        ("whh", [TD + 1, 256]),
        ("ul", [128, 2 * 1280]), ("ur", [128, 2 * 1280]),
        ("wx", [TD + 1, 1280]),
        ("th0", [TD, b]), ("tc0", [TD, b]), ("ident", [128, 128]),
    ]:
        d[name] = nc.declare_dram_parameter(name, shape, DTYPES.get(name, bf16),
                                            isOutput=False)
    d_out = nc.declare_dram_parameter("out", [b, H], f32, isOutput=True)
    d_dbg = {}
    if debug_taps:
        for name, shape in [("dbg_th", [TD, b]), ("dbg_tc", [TD, b]),
                            ("dbg_rh", [128, 2 * b]), ("dbg_rc", [128, 2 * b]),
                            ("dbg_sig", [TD, 4 * b]), ("dbg_sg", [128, 10 * b]),
                            ("dbg_psg", [128, 10 * b])]:
            d_dbg[name] = nc.declare_dram_parameter(name, shape, f32, isOutput=True)

    # Sanity-check red live ranges fit the rotating pool (bufs=3).
    red_last_use = {}
    red_birth = {}
    nred = 0
    for st in steps:
        for sym in (st["buf"], st["top"], st["sec"]):
            if sym[0] == "red":
                red_last_use[sym[1]] = max(red_last_use.get(sym[1], 0), nred)
        if st["is_red"]:
            red_birth[st["red_idx"]] = nred
            nred += 1
    if out_sym[0] == "red" and out_sym[1] in red_birth:
        red_last_use[out_sym[1]] = nred
    for r, last in red_last_use.items():
        if last - red_birth[r] > 2:
            raise NotImplementedError("red value live range too long for pool")

    with tile.TileContext(nc) as tc:
        with (
            tc.tile_pool(name="const", bufs=1) as cp,
            tc.tile_pool(name="wk", bufs=3) as wp,
            tc.tile_pool(name="pstr", bufs=2, space="PSUM") as pstr,
            tc.tile_pool(name="psgt", bufs=2, space="PSUM") as psgt,
        ):
            psmc = pstr
            sb = {}
            for name in ("tokh", "tokc", "wa", "wb", "wc", "whh", "ul", "ur",
                         "wx", "ident"):
                sb[name] = cp.tile(list(d[name].shape), DTYPES.get(name, bf16),
                                   name=f"sb_{name}")
                nc.sync.dma_start(sb[name][:], d[name].ap())

            # ping-pong state tiles; th has an extra all-ones row (bias mule)
            th_t = [cp.tile([TD + 1, b], bf16, name=f"th{i}") for i in range(2)]
            tc_t = [cp.tile([TD, b], f32, name=f"tcs{i}") for i in range(2)]
            for i in range(2):
                nc.vector.memset(th_t[i][TD:TD + 1, :], 1.0)
            nc.sync.dma_start(th_t[0][0:TD, :], d["th0"].ap())
            nc.sync.dma_start(tc_t[0][:], d["tc0"].ap())

            tokh, tokc = sb["tokh"], sb["tokc"]
            red_h, red_c = {}, {}
            last_sig = last_sg = None

            def h_rhs(sym, k):
                kind, idx = sym
                if kind == "tok":
                    return tokh[:, k * Lb + idx * b: k * Lb + (idx + 1) * b]
                return red_h[idx][:, k * b:(k + 1) * b]

            def c_view(sym):
                kind, idx = sym
                if kind == "tok":
                    v = tokc[:].rearrange("p (k l b) -> p k l b", k=2, b=b)
                    return v[:, :, idx, :]
                return red_c[idx][:].rearrange("p (k b) -> p k b", k=2)

            # ---------------- software-pipelined step emission ----------
            # The PE is in-order, so emission order controls what fills the
            # serial-chain gaps: next step's token-operand matmuls are
            # emitted inside this step's elementwise windows.
            ps_state = {}   # t -> (ps tile, [started banks])

            def track_ps(t):
                if t not in ps_state:
                    tile_ = pstr.tile([TD, 4 * b], f32, name=f"pstr_{t}",
                                      tag="pstr")
                    ps_state[t] = (tile_, set())
                return ps_state[t]

            psg_state = {}

            def tree_ps(t):
                if t not in psg_state:
                    tile_ = psgt.tile([128, 10 * b], f32, name=f"psg_{t}",
                                      tag="psg")
                    psg_state[t] = (tile_, set())
                return psg_state[t]

            from collections import Counter
            buf_counts = Counter(st["buf"] for st in steps)
            pa_cache = {}

            def pa_inject_tile(sym):
                # one-time W_A @ token projection, cached in SBUF [64, 4b]
                if sym not in pa_cache:
                    pps = psmc.tile([TD, 4 * b], f32, tag="pstr",
                                    name=f"pa_ps_{sym[1]}")
                    first = True
                    for m in range(4):
                        for k in range(2):
                            nc.tensor.matmul(
                                pps[:, m * b:(m + 1) * b],
                                sb["wa"][:, k * 256 + m * 64:
                                         k * 256 + (m + 1) * 64],
                                h_rhs(sym, k), start=first, stop=False,
                                skip_group_check=True)
                            first = False
                    pa_sb = cp.tile([TD, 4 * b], bf16, name=f"pa_sb_{sym[1]}")
                    nc.vector.tensor_copy(pa_sb[:], pps[:])
                    pa_cache[sym] = pa_sb
                return pa_cache[sym]

            def emit_track(t, want_tok, stop_last=False):
                st = steps[t]
                ps, started = track_ps(t)
                groups = [(sb["wa"], st["buf"]), (sb["wc"], st["sec"]),
                          (sb["wb"], st["top"])]
                mms = []
                for w_t, sym in groups:
                    if (sym[0] == "tok") != want_tok:
                        continue
                    if (w_t is sb["wa"] and sym[0] == "tok"
                            and buf_counts[sym] >= 4):
                        # single-matmul inject of the cached projection;
                        # must be first into the tile (sets has_written
                        # across all four windows)
                        assert 0 not in started, "inject must be first"
                        pa = pa_inject_tile(sym)
                        nc.tensor.matmul(ps[:, 0:4 * b],
                                         sb["ident"][0:TD, 0:TD], pa[:],
                                         start=True, stop=False,
                                         skip_group_check=True)
                        started.add(0)
                        continue
                    for m in range(4):
                        for k in range(2):
                            mms.append((
                                ps[:, m * b:(m + 1) * b],
                                w_t[:, k * 256 + m * 64: k * 256 + (m + 1) * 64],
                                h_rhs(sym, k)))
                for i, (o_ap, l_ap, r_ap) in enumerate(mms):
                    nc.tensor.matmul(o_ap, l_ap, r_ap,
                                     start=0 not in started,
                                     stop=stop_last and (i == len(mms) - 1),
                                     skip_group_check=True)
                    started.add(0)

            def emit_whh(t, stop=True):
                ps, started = track_ps(t)
                for m in range(4):
                    nc.tensor.matmul(ps[:, m * b:(m + 1) * b],
                                     sb["whh"][:, m * 64:(m + 1) * 64],
                                     th_t[t % 2][:], start=0 not in started,
                                     stop=stop and (m == 3),
                                     skip_group_check=True)
                    started.add(0)

            def emit_tree_u(t, want_tok):
                st = steps[t]
                psg, started = tree_ps(t)
                for w_t, sym in ((sb["ul"], st["chil"]), (sb["ur"], st["head"])):
                    if (sym[0] == "tok") != want_tok:
                        continue
                    for m in range(10):
                        bank = (m * b * 4) // 2048
                        for k in range(2):
                            nc.tensor.matmul(
                                psg[:, m * b:(m + 1) * b],
                                w_t[:, k * 1280 + m * 128: k * 1280 + (m + 1) * 128],
                                h_rhs(sym, k), start=bank not in started,
                                stop=False, skip_group_check=True)
                            started.add(bank)

            def emit_wx(t):
                psg, started = tree_ps(t)
                for m in range(10):
                    bank = (m * b * 4) // 2048
                    nc.tensor.matmul(psg[:, m * b:(m + 1) * b],
                                     sb["wx"][:, m * 128:(m + 1) * 128],
                                     th_t[(t + 1) % 2][:],
                                     start=bank not in started, stop=True,
                                     skip_group_check=True)
                    started.add(bank)

            def emit_dummies(t, n):
                # PE p-state filler: junk matmuls into a rotating psum slot
                # keep the tensor engine continuously busy so it ramps to
                # max clock.
                for dmy in range(n):
                    dps = psgt.tile([TD, 4 * b], f32, tag="dmy",
                                    name=f"dmy_{t}_{dmy}")
                    nc.tensor.matmul(dps[:], sb["ul"][0:TD, 0:TD],
                                     sb["tokh"][0:TD, 0:4 * b], start=True,
                                     stop=True, skip_group_check=True)

            def emit_track_elem(t):
                nonlocal last_sig
                cur, nxt = t % 2, (t + 1) % 2
                ps, _ = track_ps(t)
                sig = wp.tile([TD, 4 * b], f32, tag="sig", name=f"sig_{t}")
                nc.scalar.activation(sig[:], ps[:], AF.Sigmoid)
                last_sig = sig
                si, s2g = sig[:, 0:b], sig[:, b:2 * b]
                sf, so = sig[:, 2 * b:3 * b], sig[:, 3 * b:4 * b]
                At = wp.tile([TD, b], f32, tag="At", name=f"At_{t}")
                jk = wp.tile([TD, 1], f32, tag="jk", name=f"jk_{t}")
                nc.vector.affine_mul_reduce(At[:], jk[:], s2g, si, 2.0, -1.0)
                Bt = wp.tile([TD, b], f32, tag="Bt", name=f"Bt_{t}")
                nc.vector.tensor_mul(Bt[:], sf, tc_t[cur][:])
                nc.vector.tensor_add(tc_t[nxt][:], At[:], Bt[:])
                tt = wp.tile([TD, b], f32, tag="tt", name=f"tt_{t}")
                nc.scalar.activation(tt[:], tc_t[nxt][:], AF.Tanh)
                nc.vector.tensor_mul(th_t[nxt][0:TD, :], tt[:], so)

            def emit_tree_elem(t):
                nonlocal last_sg, last_psg_cp
                st = steps[t]
                psg, _ = tree_ps(t)
                sg = wp.tile([128, 10 * b], f32, tag="sg", name=f"sg_{t}")
                nc.scalar.activation(sg[:, 0:4 * b], psg[:, 0:4 * b],
                                     AF.Sigmoid)
                nc.scalar.activation(sg[:, 4 * b:10 * b],
                                     psg[:, 4 * b:10 * b], AF.Sigmoid)
                last_sg = sg
                if debug_taps:
                    psg_cp = cp.tile([128, 10 * b], f32, name=f"psgcp_{t}")
                    nc.scalar.copy(psg_cp[:], psg[:])
                    last_psg_cp = psg_cp
                s2u, sgi = sg[:, 0:2 * b], sg[:, 2 * b:4 * b]
                sgo = sg[:, 4 * b:6 * b]
                sfl, sfr = sg[:, 6 * b:8 * b], sg[:, 8 * b:10 * b]
                r3 = lambda ap: ap.rearrange("p (k b) -> p k b", k=2)
                A2 = wp.tile([128, 2 * b], f32, tag="A2", name=f"A2_{t}")
                jk2 = wp.tile([128, 1], f32, tag="jk2", name=f"jk2_{t}")
                nc.vector.affine_mul_reduce(A2[:], jk2[:], s2u, sgi, 2.0, -1.0)
                B2 = wp.tile([128, 2 * b], f32, tag="B2", name=f"B2_{t}")
                nc.vector.tensor_mul(r3(B2[:]), r3(sfl), c_view(st["chil"]))
                C2 = wp.tile([128, 2 * b], f32, tag="C2", name=f"C2_{t}")
                nc.vector.tensor_mul(r3(C2[:]), r3(sfr), c_view(st["head"]))
                S2 = wp.tile([128, 2 * b], f32, tag="S2", name=f"S2_{t}")
                nc.vector.tensor_add(S2[:], A2[:], B2[:])
                rc = wp.tile([128, 2 * b], f32, tag="rc", name=f"rc_{t}")
                nc.vector.tensor_add(rc[:], S2[:], C2[:])
                tt2 = wp.tile([128, 2 * b], f32, tag="tt2", name=f"tt2_{t}")
                nc.scalar.activation(tt2[:], rc[:], AF.Tanh)
                rh = wp.tile([128, 2 * b], bf16, tag="rh", name=f"rh_{t}")
                nc.vector.tensor_mul(rh[:], tt2[:], sgo)
                red_h[st["red_idx"]] = rh
                red_c[st["red_idx"]] = rc

            last_psg_cp = None
            nT = len(steps)
            # prologue: token matmuls of step 0
            emit_track(0, True)
            if steps[0]["is_red"]:
                emit_tree_u(0, True)
            for t in range(nT):
                st = steps[t]
                has_red_track = any(
                    sym[0] == "red" for sym in (st["buf"], st["sec"], st["top"]))
                if has_red_track:
                    # whh first (th is ready early); psum stop rides the
                    # last red-operand matmul so sigma waits only on it.
                    emit_whh(t, stop=False)
                    emit_track(t, False, stop_last=True)
                else:
                    emit_track(t, False)
                    emit_whh(t, stop=True)
                if st["is_red"]:
                    emit_tree_u(t, False)  # red-side U MMs
                # track elementwise must be EMITTED before emit_wx reads
                # th(t): Tile derives dependencies from emission order.
                # W_x goes ahead of the next-step token fills so the in-order
                # PE issues it as soon as th(t) lands instead of queueing it
                # behind ~16 filler matmuls.
                emit_track_elem(t)
                if st["is_red"]:
                    emit_wx(t)
                if t + 1 < nT:
                    emit_track(t + 1, True)   # fills tree-elem gap
                if t + 1 < nT and steps[t + 1]["is_red"]:
                    emit_tree_u(t + 1, True)  # fills tree-elem gap
                emit_dummies(t, n_dummy if st["is_red"] else max(n_dummy - 3, 0))
                if st["is_red"]:
                    emit_tree_elem(t)


            # ---- output: transpose [H, b] -> [b, H] and store ----
            if debug_taps:
                nt = len(steps)
                nc.sync.dma_start(d_dbg["dbg_th"].ap(),
                                  th_t[nt % 2][0:TD, :])
                nc.sync.dma_start(d_dbg["dbg_tc"].ap(), tc_t[nt % 2][:])
                if last_sig is not None:
                    nc.sync.dma_start(d_dbg["dbg_sig"].ap(), last_sig[:])
                if red_h:
                    rlast = max(red_h)
                    nc.sync.dma_start(d_dbg["dbg_rh"].ap(), red_h[rlast][:])
                    nc.sync.dma_start(d_dbg["dbg_rc"].ap(), red_c[rlast][:])
                    nc.sync.dma_start(d_dbg["dbg_sg"].ap(), last_sg[:])
                    nc.sync.dma_start(d_dbg["dbg_psg"].ap(), last_psg_cp[:])
            if out_sym[0] == "red" and out_sym[1] not in red_h:
                out_sym = ("tok", 0)  # truncated debug schedule: dummy out
            out_sb = wp.tile([b, H], f32, tag="out", name="out_sb")
            out_dt = bf16 if out_sym[0] == "red" else bf16
            for k in range(2):
                pot = psmc.tile([b, 128], out_dt, tag="pstr", name=f"pout_{k}")
                nc.tensor.transpose(pot[:], h_rhs(out_sym, k), sb["ident"][:])
                nc.scalar.copy(out_sb[:, k * 128:(k + 1) * 128], pot[:])
            nc.sync.dma_start(d_out.ap(), out_sb[:])

    nc.compile()
    return nc


def _build_program_g2(steps, out_sym, b, L, n_dummy=0):
    """G=2 phase-offset build: each core's batch b is split into two
    independent half-chains (bs = b//2) whose serial ops interleave on the
    engines at ~half-period offset, doubling step throughput where the
    serial dependency chain (not engine busy) is the limit.

    Token-side matmuls (operands known upfront) stay full-width and are
    shared by both chains; only the recurrent-operand matmuls (whh, W_B@red,
    U@red, wx) and all elementwise ops split per chain.  PSUM layouts keep
    the baseline (m, e) form, chain g addressing e in [g*bs, (g+1)*bs).
    """
    import concourse.bacc as bacc
    import concourse.mybir as mybir
    import concourse.tile as tile

    f32 = mybir.dt.float32
    bf16 = mybir.dt.bfloat16
    AF = mybir.ActivationFunctionType
    nc = bacc.Bacc("TRN2", target_bir_lowering=False, debug=False)
    Lb = L * b
    G = 2
    bs = b // G
    assert b % G == 0
    use_pool = bool(int(os.environ.get("KERNEL_POOL", "0")))

    d = {}
    DTYPES = {"tokc": f32, "tc0": f32}
    for name, shape in [
        ("tokh", [128, 2 * Lb]), ("tokc", [128, 2 * Lb]),
        ("wa", [128, 512]), ("wb", [128, 512]), ("wc", [128, 512]),
        ("whh", [TD + 1, 256]),
        ("ul", [128, 2 * 1280]), ("ur", [128, 2 * 1280]),
        ("wx", [TD + 1, 1280]),
        ("th0", [TD, b]), ("tc0", [TD, b]), ("ident", [128, 128]),
    ]:
        d[name] = nc.declare_dram_parameter(name, shape, DTYPES.get(name, bf16),
                                            isOutput=False)
    d_out = nc.declare_dram_parameter("out", [b, H], f32, isOutput=True)

    assert out_sym[0] == "red"

    with tile.TileContext(nc) as tc:
        with (
            tc.tile_pool(name="const", bufs=1) as cp,
            tc.tile_pool(name="wk", bufs=3) as wp,
            tc.tile_pool(name="pstr", bufs=2, space="PSUM") as pstr,
            tc.tile_pool(name="psgt", bufs=2, space="PSUM") as psgt,
        ):
            sb = {}
            for name in ("tokh", "tokc", "wa", "wb", "wc", "whh", "ul", "ur",
                         "wx", "ident"):
                sb[name] = cp.tile(list(d[name].shape), DTYPES.get(name, bf16),
                                   name=f"sb_{name}")
                nc.sync.dma_start(sb[name][:], d[name].ap())

            # combined ping-pong state (full width b); shift steps process
            # it full-width, reduce steps address per-chain slices.
            th_t = [cp.tile([TD + 1, b], bf16, name=f"th_{i}") for i in range(2)]
            tc_t = [cp.tile([TD, b], f32, name=f"tcs_{i}") for i in range(2)]
            for i in range(2):
                nc.vector.memset(th_t[i][TD:TD + 1, :], 1.0)
            nc.sync.dma_start(th_t[0][0:TD, :], d["th0"].ap())
            nc.sync.dma_start(tc_t[0][:], d["tc0"].ap())

            tokh, tokc = sb["tokh"], sb["tokc"]
            red_h = [{} for _ in range(G)]   # per chain: idx -> [128, 2*bs]
            red_c = [{} for _ in range(G)]

            def h_rhs_full(sym, k):
                kind, idx = sym
                assert kind == "tok"
                return tokh[:, k * Lb + idx * b: k * Lb + (idx + 1) * b]

            def h_rhs_g(sym, k, g):
                kind, idx = sym
                if kind == "tok":
                    base = k * Lb + idx * b + g * bs
                    return tokh[:, base: base + bs]
                return red_h[g][idx][:, k * bs:(k + 1) * bs]

            def c_view_g(sym, g):
                kind, idx = sym
                if kind == "tok":
                    v = tokc[:].rearrange("p (k l b) -> p k l b", k=2, b=b)
                    return v[:, :, idx, g * bs:(g + 1) * bs]
                return red_c[g][idx][:].rearrange("p (k b) -> p k b", k=2)

            ps_state = {}

            def track_ps(t):
                if t not in ps_state:
                    tile_ = pstr.tile([TD, 4 * b], f32, name=f"pstr_{t}",
                                      tag="pstr")
                    ps_state[t] = (tile_, set())
                return ps_state[t]

            psg_state = {}

            def tree_ps(t):
                if t not in psg_state:
                    tile_ = psgt.tile([128, 10 * b], f32, name=f"psg_{t}",
                                      tag="psg")
                    psg_state[t] = (tile_, set())
                return psg_state[t]

            from collections import Counter
            buf_counts = Counter(st["buf"] for st in steps)
            pa_cache = {}

            def pa_inject_tile(sym):
                if sym not in pa_cache:
                    pps = pstr.tile([TD, 4 * b], f32, tag="pstr",
                                    name=f"pa_ps_{sym[1]}")
                    first = True
                    for m in range(4):
                        for k in range(2):
                            nc.tensor.matmul(
                                pps[:, m * b:(m + 1) * b],
                                sb["wa"][:, k * 256 + m * 64:
                                         k * 256 + (m + 1) * 64],
                                h_rhs_full(sym, k), start=first, stop=False,
                                skip_group_check=True)
                            first = False
                    pa_sb = cp.tile([TD, 4 * b], bf16, name=f"pa_sb_{sym[1]}")
                    nc.vector.tensor_copy(pa_sb[:], pps[:])
                    pa_cache[sym] = pa_sb
                return pa_cache[sym]

            def emit_track_tok(t):
                """Shared full-width token-side track matmuls (emitted first:
                they carry the psum start flag)."""
                st = steps[t]
                ps, started = track_ps(t)
                groups = [(sb["wa"], st["buf"]), (sb["wc"], st["sec"]),
                          (sb["wb"], st["top"])]
                for w_t, sym in groups:
                    if sym[0] != "tok":
                        continue
                    if w_t is sb["wa"] and buf_counts[sym] >= 4:
                        assert 0 not in started, "inject must be first"
                        pa = pa_inject_tile(sym)
                        nc.tensor.matmul(ps[:, 0:4 * b],
                                         sb["ident"][0:TD, 0:TD], pa[:],
                                         start=True, stop=False,
                                         skip_group_check=True)
                        started.add(0)
                        continue
                    for m in range(4):
                        for k in range(2):
                            nc.tensor.matmul(
                                ps[:, m * b:(m + 1) * b],
                                w_t[:, k * 256 + m * 64: k * 256 + (m + 1) * 64],
                                h_rhs_full(sym, k), start=0 not in started,
                                stop=False, skip_group_check=True)
                            started.add(0)

            def emit_tree_tok(t):
                st = steps[t]
                psg, started = tree_ps(t)
                for w_t, sym in ((sb["ul"], st["chil"]), (sb["ur"], st["head"])):
                    if sym[0] != "tok":
                        continue
                    for m in range(10):
                        bank = (m * b * 4) // 2048
                        for k in range(2):
                            nc.tensor.matmul(
                                psg[:, m * b:(m + 1) * b],
                                w_t[:, k * 1280 + m * 128: k * 1280 + (m + 1) * 128],
                                h_rhs_full(sym, k), start=bank not in started,
                                stop=False, skip_group_check=True)
                            started.add(bank)

            def emit_whh(t, g, stop):
                ps, started = track_ps(t)
                assert 0 in started
                for m in range(4):
                    nc.tensor.matmul(
                        ps[:, m * b + g * bs: m * b + g * bs + bs],
                        sb["whh"][:, m * 64:(m + 1) * 64],
                        th_t[t % 2][:, g * bs:(g + 1) * bs], start=False,
                        stop=stop and (m == 3), skip_group_check=True)

            def emit_whh_full(t):
                ps, started = track_ps(t)
                assert 0 in started
                for m in range(4):
                    nc.tensor.matmul(
                        ps[:, m * b:(m + 1) * b],
                        sb["whh"][:, m * 64:(m + 1) * 64],
                        th_t[t % 2][:], start=False,
                        stop=(m == 3), skip_group_check=True)

            def emit_track_red(t, g):
                st = steps[t]
                ps, started = track_ps(t)
                assert st["top"][0] == "red" and 0 in started
                mms = []
                for m in range(4):
                    for k in range(2):
                        mms.append((
                            ps[:, m * b + g * bs: m * b + g * bs + bs],
                            sb["wb"][:, k * 256 + m * 64: k * 256 + (m + 1) * 64],
                            h_rhs_g(st["top"], k, g)))
                for i, (o_ap, l_ap, r_ap) in enumerate(mms):
                    nc.tensor.matmul(o_ap, l_ap, r_ap, start=False,
                                     stop=(i == len(mms) - 1),
                                     skip_group_check=True)

            def emit_tree_red(t, g):
                st = steps[t]
                psg, started = tree_ps(t)
                for w_t, sym in ((sb["ul"], st["chil"]), (sb["ur"], st["head"])):
                    if sym[0] != "red":
                        continue
                    for m in range(10):
                        bank = (m * b * 4) // 2048
                        for k in range(2):
                            nc.tensor.matmul(
                                psg[:, m * b + g * bs: m * b + g * bs + bs],
                                w_t[:, k * 1280 + m * 128: k * 1280 + (m + 1) * 128],
                                h_rhs_g(sym, k, g), start=bank not in started,
                                stop=False, skip_group_check=True)
                            started.add(bank)

            def emit_wx(t, g):
                psg, started = tree_ps(t)
                for m in range(10):
                    bank = (m * b * 4) // 2048
                    nc.tensor.matmul(
                        psg[:, m * b + g * bs: m * b + g * bs + bs],
                        sb["wx"][:, m * 128:(m + 1) * 128],
                        th_t[(t + 1) % 2][:, g * bs:(g + 1) * bs],
                        start=bank not in started, stop=True,
                        skip_group_check=True)
                    started.add(bank)

            def emit_track_elem(t, g):
                cur, nxt = t % 2, (t + 1) % 2
                ps, _ = track_ps(t)
                esl = slice(g * bs, (g + 1) * bs)
                psv = ps[:].rearrange("p (m e) -> p m e", m=4)[:, :, esl]
                sig = wp.tile([TD, 4 * bs], f32, tag=f"sig{g}", name=f"sig{g}_{t}")
                sigv = sig[:].rearrange("p (m e) -> p m e", m=4)
                nc.scalar.activation(sigv, psv, AF.Sigmoid)
                si, s2g = sig[:, 0:bs], sig[:, bs:2 * bs]
                sf, so = sig[:, 2 * bs:3 * bs], sig[:, 3 * bs:4 * bs]
                At = wp.tile([TD, bs], f32, tag=f"At{g}", name=f"At{g}_{t}")
                jk = wp.tile([TD, 1], f32, tag=f"jk{g}", name=f"jk{g}_{t}")
                nc.vector.affine_mul_reduce(At[:], jk[:], s2g, si, 2.0, -1.0)
                Bt = wp.tile([TD, bs], f32, tag=f"Bt{g}", name=f"Bt{g}_{t}")
                nc.vector.tensor_mul(Bt[:], sf, tc_t[cur][:, esl])
                nc.vector.tensor_add(tc_t[nxt][:, esl], At[:], Bt[:])
                tt = wp.tile([TD, bs], f32, tag=f"tt{g}", name=f"tt{g}_{t}")
                nc.scalar.activation(tt[:], tc_t[nxt][:, esl], AF.Tanh)
                nc.vector.tensor_mul(th_t[nxt][0:TD, esl], tt[:], so)

            def emit_track_elem_full(t):
                cur, nxt = t % 2, (t + 1) % 2
                ps, _ = track_ps(t)
                sig = wp.tile([TD, 4 * b], f32, tag="sigF", name=f"sigF_{t}")
                # split sigmoid: [i, 2g] first so affine_mul_reduce starts
                # while [f, o] is still on the Act engine
                nc.scalar.activation(sig[:, 0:2 * b], ps[:, 0:2 * b],
                                     AF.Sigmoid)
                nc.scalar.activation(sig[:, 2 * b:4 * b], ps[:, 2 * b:4 * b],
                                     AF.Sigmoid)
                si, s2g = sig[:, 0:b], sig[:, b:2 * b]
                sf, so = sig[:, 2 * b:3 * b], sig[:, 3 * b:4 * b]
                At = wp.tile([TD, b], f32, tag="AtF", name=f"AtF_{t}")
                jk = wp.tile([TD, 1], f32, tag="jkF", name=f"jkF_{t}")
                nc.vector.affine_mul_reduce(At[:], jk[:], s2g, si, 2.0, -1.0)
                Bt = wp.tile([TD, b], f32, tag="BtF", name=f"BtF_{t}")
                nc.vector.tensor_mul(Bt[:], sf, tc_t[cur][:])
                nc.vector.tensor_add(tc_t[nxt][:], At[:], Bt[:])
                tt = wp.tile([TD, b], f32, tag="ttF", name=f"ttF_{t}")
                nc.scalar.activation(tt[:], tc_t[nxt][:], AF.Tanh)
                nc.vector.tensor_mul(th_t[nxt][0:TD, :], tt[:], so)

            def emit_tree_elem(t, g):
                st = steps[t]
                psg, _ = tree_ps(t)
                psv = psg[:].rearrange("p (m e) -> p m e", m=10)[:, :, g * bs:(g + 1) * bs]
                sg = wp.tile([128, 10 * bs], f32, tag=f"sg{g}", name=f"sg{g}_{t}")
                sgv = sg[:].rearrange("p (m e) -> p m e", m=10)
                nc.scalar.activation(sgv, psv, AF.Sigmoid)
                s2u, sgi = sg[:, 0:2 * bs], sg[:, 2 * bs:4 * bs]
                sgo = sg[:, 4 * bs:6 * bs]
                sfl, sfr = sg[:, 6 * bs:8 * bs], sg[:, 8 * bs:10 * bs]
                r3 = lambda ap: ap.rearrange("p (k b) -> p k b", k=2)
                A2 = wp.tile([128, 2 * bs], f32, tag=f"A2{g}", name=f"A2{g}_{t}")
                jk2 = wp.tile([128, 1], f32, tag=f"jk2{g}", name=f"jk2{g}_{t}")
                nc.vector.affine_mul_reduce(A2[:], jk2[:], s2u, sgi, 2.0, -1.0)
                mul_eng = nc.gpsimd if use_pool else nc.vector
                B2 = wp.tile([128, 2 * bs], f32, tag=f"B2{g}", name=f"B2{g}_{t}")
                mul_eng.tensor_mul(r3(B2[:]), r3(sfl), c_view_g(st["chil"], g))
                C2 = wp.tile([128, 2 * bs], f32, tag=f"C2{g}", name=f"C2{g}_{t}")
                mul_eng.tensor_mul(r3(C2[:]), r3(sfr), c_view_g(st["head"], g))
                S2 = wp.tile([128, 2 * bs], f32, tag=f"S2{g}", name=f"S2{g}_{t}")
                nc.vector.tensor_add(S2[:], A2[:], B2[:])
                rc = wp.tile([128, 2 * bs], f32, tag=f"rc{g}", name=f"rc{g}_{t}")
                nc.vector.tensor_add(rc[:], S2[:], C2[:])
                tt2 = wp.tile([128, 2 * bs], f32, tag=f"tt2{g}", name=f"tt2{g}_{t}")
                nc.scalar.activation(tt2[:], rc[:], AF.Tanh)
                rh = wp.tile([128, 2 * bs], bf16, tag=f"rh{g}", name=f"rh{g}_{t}")
                nc.vector.tensor_mul(rh[:], tt2[:], sgo)
                red_h[g][st["red_idx"]] = rh
                red_c[g][st["red_idx"]] = rc

            nT = len(steps)
            # prologue: shared token matmuls of step 0
            emit_track_tok(0)
            if steps[0]["is_red"]:
                emit_tree_tok(0)
            for t in range(nT):
                st = steps[t]
                has_red_top = st["top"][0] == "red"
                if not st["is_red"] and not has_red_top:
                    # shift step: full-width (chain split only pays where
                    # tree work overlaps; half-size ops are overhead-bound)
                    emit_whh_full(t)
                    emit_track_elem_full(t)
                    if t + 1 < nT:
                        emit_track_tok(t + 1)
                        if steps[t + 1]["is_red"]:
                            emit_tree_tok(t + 1)
                    continue
                for g in range(G):
                    emit_whh(t, g, stop=not has_red_top)
                    if has_red_top:
                        emit_track_red(t, g)
                    if st["is_red"]:
                        emit_tree_red(t, g)
                    emit_track_elem(t, g)
                    if st["is_red"]:
                        emit_wx(t, g)
                    if g == 0 and t + 1 < nT:
                        emit_track_tok(t + 1)
                        if steps[t + 1]["is_red"]:
                            emit_tree_tok(t + 1)
                    if st["is_red"]:
                        emit_tree_elem(t, g)

            # ---- output: transpose per chain [H, bs] -> [bs, H] and store --
            out_sb = wp.tile([b, H], f32, tag="out", name="out_sb")
            for g in range(G):
                rh_last = red_h[g][out_sym[1]]
                for k in range(2):
                    pot = pstr.tile([bs, 128], bf16, tag="pstr",
                                    name=f"pout{g}_{k}")
                    nc.tensor.transpose(pot[:], rh_last[:, k * bs:(k + 1) * bs],
                                        sb["ident"][:])
                    nc.scalar.copy(out_sb[g * bs:(g + 1) * bs,
                                          k * 128:(k + 1) * 128], pot[:])
            nc.sync.dma_start(d_out.ap(), out_sb[:])

    nc.compile()
    return nc


_PROGRAM_CACHE = {}


def _get_program(codes_key, b, L, steps, out_sym):
    nd = int(os.environ.get("KERNEL_NDUMMY", "0"))
    g2 = int(os.environ.get("KERNEL_G2", "1"))
    key = (codes_key, b, L, nd, g2)
    if key not in _PROGRAM_CACHE:
        build = _build_program_g2 if g2 else _build_program
        _PROGRAM_CACHE[key] = build(steps, out_sym, b, L, n_dummy=nd)
    return _PROGRAM_CACHE[key]


# ------------------------------------------------------------------ kernel --

def kernel(**inputs) -> np.ndarray:
    from concourse.bass_utils import run_bass_kernel_spmd

    tokens_h = np.asarray(inputs["tokens_h"], np.float32)
    tokens_c = np.asarray(inputs["tokens_c"], np.float32)
    transitions = np.asarray(inputs["transitions"])
    th0 = np.asarray(inputs["th0"], np.float32)
    tc0 = np.asarray(inputs["tc0"], np.float32)
    B, L, Hn = tokens_h.shape
    assert Hn == H and B % N_CORES == 0
    b = B // N_CORES

    steps, out_sym = derive_schedule(transitions, L)
    codes_key = tuple(int(c) for c in transitions[0])
    nc = _get_program(codes_key, b, L, steps, out_sym)

    w = prep_weights(inputs["W_x"], inputs["U_r"], inputs["U_l"], inputs["b_l"],
                     inputs["W_ih"], inputs["W_hh"], inputs["b_ih"], inputs["b_hh"])
    in_maps = []
    for core in range(N_CORES):
        sl = slice(core * b, (core + 1) * b)
        m = dict(w)
        m["tokh"] = prep_tokens(tokens_h[sl], BF16)
        m["tokc"] = prep_tokens(tokens_c[sl])
        m["th0"] = np.ascontiguousarray(th0[sl].T.astype(BF16))
        m["tc0"] = np.ascontiguousarray(tc0[sl].T)
        in_maps.append(m)

    trace = bool(int(os.environ.get("KERNEL_TRACE", "0")))
    res = run_bass_kernel_spmd(nc, in_maps, list(range(N_CORES)), trace=trace)
    if trace:
        kernel.last_exec_time_ns = res.exec_time_ns
        kernel.last_results = res
    out = np.concatenate([res.results[i]["out"] for i in range(N_CORES)], axis=0)
    return np.ascontiguousarray(out, dtype=np.float32)

